# revision 36
# baseline (speedup 1.0000x reference)
"""Trainium2 Bass kernel for nn_BBoxVAE: 24-step conditional-VAE LSTM over batch 32768.

Strategy: pure data-parallel over 8 NeuronCores (4096 rows/core). On-device
compute is feature-major (features on SBUF partitions, batch on the free dim),
fp32 data with float32r matmuls on the PE. The loop-invariant label branch and
the one-hot/i2 branch are folded on the host; the encoder's gt-only prefix is
hoisted into a pre-pass (einter, via a DRAM round-trip). Sigmoids are computed
as tanh (single ACT table set: exp_and_others). Inputs eps/bbox/label are
transposed on the host; outputs are produced transposed and untransposed on
the host.
"""
import numpy as np

import concourse.bass as bass
import concourse.mybir as mybir
import concourse.tile as tile
from concourse import bacc

dt = mybir.dt
AF = mybir.ActivationFunctionType
ALU = mybir.AluOpType

B, L, H, Z = 32768, 24, 128, 32
N_CORES = 8
CH = 512  # batch columns per matmul / elementwise op

LN_HALF = float(np.log(0.5))

_W_SHAPES = dict(
    cw1=(L, H), cw2=(H, H), cwo1=(H, H), cw5h=(H, H), cwo3=(H, H),
    php=(H, 2 * Z), Wz=(2 * Z, H), ewm=(Z, Z),
    dw1a=(H, H), dw1b=(Z, H), dw2=(H, 64), dw3=(64, 4),
    lk4h=(4, 4 * H), lrh=(H, 4 * H), ew1=(4, H), ew23a=(H, Z),
    eye32=(Z, Z),
)

# bvec columns
BV_CB5, BV_PHINT, BV_DB1, BV_DB2, BV_DB3H, BV_DELTA, BV_EBM, BV_CB1, \
    BV_CB2, BV_EB1, BV_EIB, BV_XSCALE, BV_XBIAS, BV_CKL, BV_XBIAS3, \
    BV_XBIAS2 = range(16)
NBV = 16


def _build(bs, reps=1):
    """Emit the per-core Bass program for a batch shard of `bs` rows.

    reps>1 repeats the recurrence (timing experiments only)."""
    nchunk = bs // CH
    nc = bacc.Bacc(None, target_bir_lowering=False)

    lsT = nc.dram_tensor("lsT", [L, bs], dt.float32, kind="ExternalInput")
    gtT = nc.dram_tensor("gtT", [L, 4, bs], dt.float32, kind="ExternalInput")
    epsT = nc.dram_tensor("epsT", [L, Z, bs], dt.float32, kind="ExternalInput")
    W = {k: nc.dram_tensor("w_" + k, list(sh), dt.float32, kind="ExternalInput")
         for k, sh in _W_SHAPES.items()}
    bvec = nc.dram_tensor("bvec", [H, NBV], dt.float32, kind="ExternalInput")
    gbias = nc.dram_tensor("gbias", [H, L * 4], dt.float32, kind="ExternalInput")
    brow = nc.dram_tensor("brow", [H, L], dt.float32, kind="ExternalInput")
    klT = nc.dram_tensor("klT", [L, Z, bs], dt.float32, kind="ExternalOutput")
    tbT = nc.dram_tensor("tbT", [L, 4, bs], dt.float32, kind="ExternalOutput")

    with tile.TileContext(nc) as tc:
        with tc.tile_pool(name="const", bufs=1) as cp, \
             tc.tile_pool(name="state", bufs=1) as sp, \
             tc.tile_pool(name="work", bufs=2) as wp, \
             tc.tile_pool(name="ps", bufs=2, space="PSUM") as pp, \
             tc.tile_pool(name="psD", bufs=1, space="PSUM") as pd, \
             tc.tile_pool(name="psC", bufs=1, space="PSUM") as pc, \
             tc.tile_pool(name="psF", bufs=1, space="PSUM") as pf, \
             tc.tile_pool(name="psG", bufs=3, space="PSUM") as pg, \
             tc.tile_pool(name="dram", bufs=1, space="DRAM") as dp:

            # ---- constants into SBUF (weights rounded to f32r via gpsimd cast DMA)
            wt = {}
            for k, sh in _W_SHAPES.items():
                t = cp.tile(list(sh), dt.float32r, tag="w_" + k)
                nc.gpsimd.dma_start(out=t[:], in_=W[k][:])
                wt[k] = t
            bv = cp.tile([H, NBV], dt.float32, tag="bvec")
            nc.sync.dma_start(out=bv[:], in_=bvec[:])
            gb = cp.tile([H, L * 4], dt.float32, tag="gbias")
            nc.sync.dma_start(out=gb[:], in_=gbias[:])
            br = cp.tile([H, L], dt.float32, tag="brow")
            nc.sync.dma_start(out=br[:], in_=brow[:])

            # ---- persistent state, one tile per chunk (independent dep chains)
            hS = [sp.tile([H, CH], dt.float32r, tag=f"hS{c}", name=f"hS{c}")
                  for c in range(nchunk)]
            cS = [sp.tile([H, CH], dt.float32, tag=f"cS{c}", name=f"cS{c}")
                  for c in range(nchunk)]
            c1 = [sp.tile([H, CH], dt.float32, tag=f"c1{c}", name=f"c1{c}")
                  for c in range(nchunk)]
            for c in range(nchunk):
                nc.vector.memset(hS[c][:].bitcast(dt.float32), 0.0)
                nc.vector.memset(cS[c][:], 0.0)

            einter_d = dp.tile([L, Z, bs], dt.float32, tag="einter")

            # ---- hoist: i1 -> c1 (label branch)
            for c in range(nchunk):
                sl = slice(c * CH, (c + 1) * CH)
                lst = wp.tile([L, CH], dt.float32r, tag="lst")
                nc.gpsimd.dma_start(out=lst[:], in_=lsT[:, sl])
                pA = pp.tile([H, CH], dt.float32, tag="ps")
                nc.tensor.matmul(pA[:], wt["cw1"][:], lst[:], start=True, stop=True)
                i1a = wp.tile([H, CH], dt.float32r, tag="i1a")
                nc.scalar.activation(i1a[:], pA[:], AF.Relu,
                                     bias=bv[:, BV_CB1:BV_CB1 + 1], scale=1.0)
                pB = pp.tile([H, CH], dt.float32, tag="ps")
                nc.tensor.matmul(pB[:], wt["cw2"][:], i1a[:], start=True, stop=True)
                i1b = wp.tile([H, CH], dt.float32r, tag="i1b")
                nc.scalar.activation(i1b[:], pB[:], AF.Relu,
                                     bias=bv[:, BV_CB2:BV_CB2 + 1], scale=1.0)
                pD = pp.tile([H, CH], dt.float32, tag="ps")
                nc.tensor.matmul(pD[:], wt["cwo1"][:], i1b[:], start=True, stop=True)
                nc.vector.tensor_copy(c1[c][:], pD[:])

            # ---- hoist: einter (gt-only encoder prefix), via DRAM
            for t in range(L):
                for c in range(nchunk):
                    sl = slice(c * CH, (c + 1) * CH)
                    gtc = wp.tile([4, CH], dt.float32r, tag="gtc")
                    nc.gpsimd.dma_start(out=gtc[:], in_=gtT[t, :, sl])
                    pJ = pp.tile([H, CH], dt.float32, tag="ps")
                    nc.tensor.matmul(pJ[:], wt["ew1"][:], gtc[:], start=True, stop=True)
                    e1 = wp.tile([H, CH], dt.float32r, tag="e1")
                    nc.scalar.activation(e1[:], pJ[:], AF.Relu,
                                         bias=bv[:, BV_EB1:BV_EB1 + 1], scale=1.0)
                    pL = pc.tile([64, CH], dt.float32, tag="psC")
                    nc.tensor.matmul(pL[0:Z, :], wt["ew23a"][:], e1[:],
                                     start=True, stop=True)
                    ei = wp.tile([Z, CH], dt.float32, tag="ei")
                    nc.vector.tensor_scalar(ei[:], pL[0:Z, :],
                                            bv[:Z, BV_EIB:BV_EIB + 1], None, ALU.add)
                    nc.sync.dma_start(out=einter_d[t, :, sl], in_=ei[:])

            # ---- recurrence
            for t in [t for _ in range(reps) for t in range(L)]:
                for c in range(nchunk):
                    sl = slice(c * CH, (c + 1) * CH)
                    # i3 = relu(h @ cw5 + cb5)  (h = hS/2 folded into cw5h)
                    pA = pp.tile([H, CH], dt.float32, tag="ps")
                    nc.tensor.matmul(pA[:], wt["cw5h"][:], hS[c][:],
                                     start=True, stop=True)
                    i3 = wp.tile([H, CH], dt.float32r, tag="i3")
                    nc.vector.tensor_scalar(i3[:], pA[:], bv[:, BV_CB5:BV_CB5 + 1],
                                            0.0, ALU.add, op1=ALU.max)
                    # cond = i3 @ cwo3 + brow[t] + c1
                    pB = pp.tile([H, CH], dt.float32, tag="ps")
                    nc.tensor.matmul(pB[:], wt["cwo3"][:], i3[:], start=True, stop=True)
                    cond = wp.tile([H, CH], dt.float32r, tag="cond")
                    nc.vector.scalar_tensor_tensor(cond[:], pB[:], br[:, t:t + 1],
                                                   c1[c][:], ALU.add, ALU.add)
                    # phint = relu([inter_pre + einter ; ph_pre + pb1])
                    # einter is accumulated on the PE via an identity matmul
                    pC = pc.tile([64, CH], dt.float32, tag="psC")
                    nc.tensor.matmul(pC[:], wt["php"][:], cond[:],
                                     start=True, stop=False, skip_group_check=True)
                    eic = wp.tile([Z, CH], dt.float32r, tag="eic")
                    nc.gpsimd.dma_start(out=eic[:], in_=einter_d[t, :, sl])
                    nc.tensor.matmul(pC[0:Z, :], wt["eye32"][:], eic[:],
                                     start=False, stop=True, skip_group_check=True)
                    phint = wp.tile([64, CH], dt.float32r, tag="phint")
                    nc.vector.tensor_scalar(phint[:], pC[:],
                                            bv[:64, BV_PHINT:BV_PHINT + 1],
                                            0.0, ALU.add, op1=ALU.max)
                    # z-block: psD = [d | zv | f1 | zvc] (raw, biases downstream)
                    pD = pd.tile([H, CH], dt.float32, tag="psD")
                    nc.tensor.matmul(pD[:], wt["Wz"][:], phint[:], start=True, stop=True)
                    # zm reuses the psC bank (zm-mm needs phint, so the wait is free)
                    pZm = pc.tile([Z, CH], dt.float32, tag="psC")
                    nc.tensor.matmul(pZm[:], wt["ewm"][:], phint[0:Z, :],
                                     start=True, stop=True)
                    # X64 = [E=.5exp(f1_b); e2'=.5exp(-zvc_b)]; e3 = exp(.5 zv_b)
                    X64 = wp.tile([64, CH], dt.float32, tag="X64")
                    nc.scalar.activation(X64[:], pD[2 * Z:4 * Z, :], AF.Exp,
                                         bias=bv[:64, BV_XBIAS:BV_XBIAS + 1],
                                         scale=bv[:64, BV_XSCALE:BV_XSCALE + 1])
                    e3 = wp.tile([Z, CH], dt.float32, tag="e3")
                    nc.scalar.activation(e3[:], pD[Z:2 * Z, :], AF.Exp,
                                         bias=bv[:Z, BV_XBIAS3:BV_XBIAS3 + 1],
                                         scale=0.5)
                    # base-32 lane: d2s, m2, k1, klv all live at partitions 32:64
                    d64 = wp.tile([64, CH], dt.float32, tag="d64")
                    nc.scalar.activation(d64[Z:2 * Z, :], pD[0:Z, :], AF.Square,
                                         bias=bv[:Z, BV_DELTA:BV_DELTA + 1], scale=1.0)
                    k64 = wp.tile([64, CH], dt.float32, tag="k64")
                    nc.vector.scalar_tensor_tensor(k64[Z:2 * Z, :],
                                                   pD[2 * Z:3 * Z, :], -0.5,
                                                   X64[0:Z, :], ALU.mult, ALU.add)
                    m64 = wp.tile([64, CH], dt.float32, tag="m64")
                    nc.gpsimd.tensor_tensor(m64[Z:2 * Z, :], d64[Z:2 * Z, :],
                                            X64[Z:2 * Z, :], ALU.mult)
                    klv = wp.tile([64, CH], dt.float32, tag="klv")
                    nc.gpsimd.tensor_tensor(klv[Z:2 * Z, :], m64[Z:2 * Z, :],
                                            k64[Z:2 * Z, :], ALU.add)
                    nc.sync.dma_start(out=klT[t, :, sl], in_=klv[Z:2 * Z, :])
                    # z = zm_b + e3 * eps
                    epc = wp.tile([Z, CH], dt.float32, tag="epc")
                    nc.sync.dma_start(out=epc[:], in_=epsT[t, :, sl])
                    p1v = wp.tile([Z, CH], dt.float32, tag="p1v")
                    nc.gpsimd.tensor_tensor(p1v[:], e3[:], epc[:], ALU.mult)
                    z_ = wp.tile([Z, CH], dt.float32r, tag="z_")
                    nc.vector.scalar_tensor_tensor(z_[:], pZm[:],
                                                   bv[:Z, BV_EBM:BV_EBM + 1],
                                                   p1v[:], ALU.add, ALU.add)
                    # decoder
                    pE = pp.tile([H, CH], dt.float32, tag="ps")
                    nc.tensor.matmul(pE[:], wt["dw1a"][:], cond[:],
                                     start=True, stop=False)
                    nc.tensor.matmul(pE[:], wt["dw1b"][:], z_[:],
                                     start=False, stop=True)
                    d1 = wp.tile([H, CH], dt.float32r, tag="d1")
                    nc.vector.tensor_scalar(d1[:], pE[:], bv[:, BV_DB1:BV_DB1 + 1],
                                            0.0, ALU.add, op1=ALU.max)
                    pF = pf.tile([64, CH], dt.float32, tag="psF")
                    nc.tensor.matmul(pF[:], wt["dw2"][:], d1[:],
                                     start=True, stop=True)
                    d2dec = wp.tile([64, CH], dt.float32r, tag="d2dec")
                    nc.scalar.activation(d2dec[:], pF[:], AF.Relu,
                                         bias=bv[:64, BV_DB2:BV_DB2 + 1], scale=1.0)
                    # d3 reuses the psF bank (d3-mm needs d2dec, so the wait is free)
                    pD3 = pf.tile([4, CH], dt.float32, tag="psF")
                    nc.tensor.matmul(pD3[:], wt["dw3"][:], d2dec[:],
                                     start=True, stop=True)
                    tb = wp.tile([4, CH], dt.float32r, tag="tb")
                    nc.scalar.activation(tb[:], pD3[:], AF.Tanh,
                                         bias=bv[:4, BV_DB3H:BV_DB3H + 1], scale=0.5)
                    nc.sync.dma_start(out=tbT[t, :, sl], in_=tb[:].bitcast(dt.float32))
                    # LSTM gates (i,f,g,o); sig(x)=.5+.5tanh(x/2) folded
                    tg = []
                    for g in range(4):
                        pG = pg.tile([H, CH], dt.float32, tag="gates")
                        nc.tensor.matmul(pG[:], wt["lrh"][:, g * H:(g + 1) * H],
                                         hS[c][:], start=True, stop=False)
                        nc.tensor.matmul(pG[:], wt["lk4h"][:, g * H:(g + 1) * H],
                                         tb[:], start=False, stop=True)
                        tgt = wp.tile([H, CH], dt.float32, tag=f"tg{g}")
                        col = t * 4 + g
                        nc.scalar.activation(tgt[:], pG[:], AF.Tanh,
                                             bias=gb[:, col:col + 1],
                                             scale=1.0 if g == 2 else 0.5)
                        tg.append(tgt)
                    # c' = .5*(1+tf)c' + (1+ti)tg ; tc = tanh(.5 c');
                    # h' = (1+to)tc  (h'=2h, c'=2c)
                    u1 = wp.tile([H, CH], dt.float32, tag="u1")
                    nc.vector.scalar_tensor_tensor(u1[:], tg[1][:], 1.0, cS[c][:],
                                                   ALU.add, ALU.mult)
                    u2 = wp.tile([H, CH], dt.float32, tag="u2")
                    nc.gpsimd.tensor_tensor(u2[:], tg[0][:], tg[2][:], ALU.mult)
                    nc.gpsimd.tensor_tensor(u2[:], u2[:], tg[2][:], ALU.add)
                    nc.vector.scalar_tensor_tensor(cS[c][:], u1[:], 0.5, u2[:],
                                                   ALU.mult, ALU.add)
                    tcil = wp.tile([H, CH], dt.float32, tag="tcil")
                    nc.scalar.activation(tcil[:], cS[c][:], AF.Tanh,
                                         bias=0.0, scale=0.5)
                    nc.vector.scalar_tensor_tensor(hS[c][:], tg[3][:], 1.0, tcil[:],
                                                   ALU.add, ALU.mult)

    nc.finalize()
    return nc


def _host_prep(inputs):
    """Fold weights/biases on the host. Returns (weight_map, C_kl)."""
    f4 = np.float32
    g = {k: np.asarray(v, np.float64) for k, v in inputs.items()
         if k not in ("label_set", "bbox_input", "eps")}

    relu = lambda x: np.maximum(x, 0.0)
    eye = np.eye(L)
    i2a = relu(relu(eye @ g["cw3"] + g["cb3"]) @ g["cw4"] + g["cb4"])  # [L,H]
    browA = (i2a @ g["cwo"][H:2 * H] + g["cbo"])                        # [L,H]

    wm = {}
    wm["w_cw1"] = g["cw1"]
    wm["w_cw2"] = g["cw2"]
    wm["w_cwo1"] = g["cwo"][0:H]
    wm["w_cw5h"] = 0.5 * g["cw5"]
    wm["w_cwo3"] = g["cwo"][2 * H:3 * H]
    # phint layout [inter_pre(0:32); ph_pre(32:64)] -> one matmul
    wm["w_php"] = np.concatenate([g["ew3"][H:2 * H], g["pw1"]], axis=1)  # [H,64]
    ZZ = np.zeros((Z, Z))
    # columns: [d | zv | f1 | zvc]; rows: [inter(0:32); ph(32:64)]
    wm["w_Wz"] = np.block([[g["ewm"], g["ewv"], g["ewv"], ZZ],
                           [-g["pwm"], ZZ, -g["pwv"], g["pwv"]]])  # [64, 4Z]
    wm["w_ewm"] = g["ewm"]
    wm["w_dw1a"] = g["dw1"][0:H]
    wm["w_dw1b"] = g["dw1"][H:H + Z]
    wm["w_dw2"] = g["dw2"]
    wm["w_dw3"] = g["dw3"]
    wm["w_lk4h"] = 0.5 * g["lk"][L:L + 4]
    wm["w_lrh"] = 0.5 * g["lr"]
    wm["w_ew1"] = g["ew1"]
    wm["w_ew23a"] = g["ew2"] @ g["ew3"][0:H]
    wm["w_eye32"] = np.eye(Z)

    bvec = np.zeros((H, NBV))
    bvec[:, BV_CB5] = g["cb5"]
    bvec[Z:2 * Z, BV_PHINT] = g["pb1"]          # [inter(no bias); ph+pb1]
    bvec[:, BV_DB1] = g["db1"]
    bvec[:64, BV_DB2] = g["db2"]
    bvec[:4, BV_DB3H] = 0.5 * g["db3"]
    bvec[:Z, BV_DELTA] = g["ebm"] - g["pbm"]
    bvec[:Z, BV_EBM] = g["ebm"]
    bvec[:, BV_CB1] = g["cb1"]
    bvec[:, BV_CB2] = g["cb2"]
    bvec[:, BV_EB1] = g["eb1"]
    bvec[:Z, BV_EIB] = g["eb2"] @ g["ew3"][0:H] + g["eb3"]
    kappa = g["ebv"] - g["pbv"]
    # X64 ACT reads [f1; zvc]: scales [+1; -1]
    bvec[0:Z, BV_XSCALE] = 1.0
    bvec[Z:2 * Z, BV_XSCALE] = -1.0
    bvec[0:Z, BV_XBIAS] = kappa + LN_HALF            # E  = exp(f1_raw + k + ln.5)
    bvec[Z:2 * Z, BV_XBIAS] = -g["pbv"] + LN_HALF    # e2' = exp(-zvc - pbv + ln.5)
    bvec[0:Z, BV_XBIAS3] = 0.5 * g["ebv"]            # e3 = exp(.5*zv_raw + .5*ebv)
    # kl constant C = -0.5*kappa - 0.5 is added on the host after gather
    ckl = (-0.5 * kappa - 0.5).astype(f4)

    base = g["lb"][None, :] + g["lk"][0:L] + 0.5 * g["lk"][L:L + 4].sum(0)  # [L,512]
    gbias = np.zeros((H, L * 4))
    for t in range(L):
        for gi in range(4):
            sc = 1.0 if gi == 2 else 0.5
            gbias[:, t * 4 + gi] = sc * base[t, gi * H:(gi + 1) * H]

    wm = {k: np.ascontiguousarray(v, dtype=f4) for k, v in wm.items()}
    wm["bvec"] = np.ascontiguousarray(bvec, dtype=f4)
    wm["gbias"] = np.ascontiguousarray(gbias, dtype=f4)
    wm["brow"] = np.ascontiguousarray(browA.T, dtype=f4)  # [H, L]
    return wm, ckl


class Runner:
    """Compile once, execute many times (for timing)."""

    def __init__(self, nc, n_cores):
        import jax
        from jax.sharding import Mesh, PartitionSpec
        from jax.experimental.shard_map import shard_map
        from concourse import bass2jax

        bass2jax.install_neuronx_cc_hook()
        self.n_cores = n_cores
        partition_name = (nc.partition_id_tensor.name
                          if nc.partition_id_tensor else None)
        in_names, out_names, out_avals, zero_outs = [], [], [], []
        for alloc in nc.m.functions[0].allocations:
            if not isinstance(alloc, mybir.MemoryLocationSet):
                continue
            name = alloc.memorylocations[0].name
            if alloc.kind == "ExternalInput":
                if name != partition_name:
                    in_names.append(name)
            elif alloc.kind == "ExternalOutput":
                shape = tuple(alloc.tensor_shape)
                dtype = mybir.dt.np(alloc.dtype)
                out_names.append(name)
                out_avals.append(jax.core.ShapedArray(shape, dtype))
                zero_outs.append(np.zeros(shape, dtype))
        self.in_names, self.out_names = in_names, out_names
        self.out_avals, self.zero_outs = out_avals, zero_outs
        n_params, n_outs = len(in_names), len(out_names)
        self.n_params = n_params

        def _body(*args):
            operands = list(args)
            if partition_name is not None:
                operands.append(bass2jax.partition_id_tensor())
            outs = bass2jax._bass_exec_p.bind(
                *operands,
                out_avals=tuple(out_avals),
                in_names=tuple(in_names + out_names
                               + ([partition_name] if partition_name else [])),
                out_names=tuple(out_names),
                lowering_input_output_aliases=(),
                sim_require_finite=True,
                sim_require_nnan=True,
                nc=nc,
            )
            return tuple(outs)

        donate = tuple(range(n_params, n_params + n_outs))
        if n_cores == 1:
            self.fn = jax.jit(_body, donate_argnums=donate, keep_unused=True)
            self.mesh = None
        else:
            devices = jax.devices()[:n_cores]
            mesh = Mesh(np.asarray(devices), ("core",))
            in_specs = (PartitionSpec("core"),) * (n_params + n_outs)
            out_specs = (PartitionSpec("core"),) * n_outs
            self.fn = jax.jit(
                shard_map(_body, mesh=mesh, in_specs=in_specs,
                          out_specs=out_specs, check_rep=False),
                donate_argnums=donate, keep_unused=True)
            self.mesh = mesh

    def run(self, in_maps):
        n = self.n_cores
        assert len(in_maps) == n
        concat_in = [np.concatenate([np.asarray(in_maps[c][k])
                                     for c in range(n)], axis=0)
                     for k in self.in_names]
        concat_zeros = [np.zeros((n * z.shape[0], *z.shape[1:]), z.dtype)
                        for z in self.zero_outs]
        out = self.fn(*concat_in, *concat_zeros)
        return [
            {name: np.asarray(out[i]).reshape(n, *self.out_avals[i].shape)[c]
             for i, name in enumerate(self.out_names)}
            for c in range(n)
        ]

    def bench(self, in_maps, iters=5):
        """Device-resident inputs; time only dispatch+execute."""
        import time
        import jax
        from jax.sharding import NamedSharding, PartitionSpec

        n = self.n_cores
        concat_in = [np.concatenate([np.asarray(in_maps[c][k])
                                     for c in range(n)], axis=0)
                     for k in self.in_names]
        if self.mesh is not None:
            sh = NamedSharding(self.mesh, PartitionSpec("core"))
            dev_in = [jax.device_put(a, sh) for a in concat_in]
        else:
            dev_in = [jax.device_put(a) for a in concat_in]
        jax.block_until_ready(dev_in)
        times = []
        for _ in range(iters):
            zeros = [np.zeros((n * z.shape[0], *z.shape[1:]), z.dtype)
                     for z in self.zero_outs]
            if self.mesh is not None:
                dev_zeros = [jax.device_put(a, sh) for a in zeros]
            else:
                dev_zeros = [jax.device_put(a) for a in zeros]
            jax.block_until_ready(dev_zeros)
            t0 = time.perf_counter()
            out = self.fn(*dev_in, *dev_zeros)
            jax.block_until_ready(out)
            times.append(time.perf_counter() - t0)
            del out
        return times


_CACHE = {}


def _get_runner(bs, n_cores):
    key = (bs, n_cores)
    if key not in _CACHE:
        nc = _build(bs)
        _CACHE[key] = Runner(nc, n_cores)
    return _CACHE[key]


def _make_in_maps(inputs, n_cores):
    wm, ckl = _host_prep(inputs)
    f4 = np.float32
    ls = np.asarray(inputs["label_set"], f4)
    bb = np.asarray(inputs["bbox_input"], f4)
    ep = np.asarray(inputs["eps"], f4)
    bs = ls.shape[0] // n_cores
    in_maps = []
    for c in range(n_cores):
        s = slice(c * bs, (c + 1) * bs)
        m = dict(wm)
        m["lsT"] = np.ascontiguousarray(ls[s].T)
        m["gtT"] = np.ascontiguousarray(bb[s].transpose(1, 2, 0))
        m["epsT"] = np.ascontiguousarray(ep[s].transpose(1, 2, 0))
        in_maps.append(m)
    return in_maps, bs, ckl


def _assemble(results, ckl):
    bbox_parts, kl_parts = [], []
    for r in results:
        tb = r["tbT"]                      # [L,4,bs]
        kl = r["klT"]                      # [L,Z,bs]
        bbox_parts.append((0.5 + 0.5 * tb).transpose(2, 0, 1))
        kl_parts.append(kl.transpose(2, 0, 1) + ckl[None, None, :])
    return (np.ascontiguousarray(np.concatenate(bbox_parts, 0), dtype=np.float32),
            np.ascontiguousarray(np.concatenate(kl_parts, 0), dtype=np.float32))


def kernel(**inputs):
    n_cores = N_CORES
    in_maps, bs, ckl = _make_in_maps(inputs, n_cores)
    runner = _get_runner(bs, n_cores)
    results = runner.run(in_maps)
    return _assemble(results, ckl)


# revision 37
# speedup vs baseline: 2.3489x; 2.3489x over previous
"""Trainium2 Bass kernel for nn_BBoxVAE: 24-step conditional-VAE LSTM over batch 32768.

Strategy: pure data-parallel over 8 NeuronCores (4096 rows/core). On-device
compute is feature-major (features on SBUF partitions, batch on the free dim),
fp32 data with float32r matmuls on the PE. The loop-invariant label branch and
the one-hot/i2 branch are folded on the host; the encoder's gt-only prefix is
hoisted into a pre-pass (einter, via a DRAM round-trip). Sigmoids are computed
as tanh (single ACT table set: exp_and_others). Inputs eps/bbox/label are
transposed on the host; outputs are produced transposed and untransposed on
the host.
"""
import numpy as np

import concourse.bass as bass
import concourse.mybir as mybir
import concourse.tile as tile
from concourse import bacc

dt = mybir.dt
AF = mybir.ActivationFunctionType
ALU = mybir.AluOpType

B, L, H, Z = 32768, 24, 128, 32
N_CORES = 8
CH = 512  # batch columns per matmul / elementwise op

LN_HALF = float(np.log(0.5))

_W_SHAPES = dict(
    cw1=(L, H), cw2=(H, H), cwo1=(H, H), cw5h=(H, H), cwo3=(H, H),
    php=(H, 2 * Z), Wz=(2 * Z, H), ewm=(Z, Z),
    dw1a=(H, H), dw1b=(Z, H), dw2=(H, 64), dw3=(64, 4),
    lk4h=(4, 4 * H), lrh=(H, 4 * H), ew1=(4, H), ew23a=(H, Z),
    eye32=(Z, Z),
)

# bvec columns
BV_CB5, BV_PHINT, BV_DB1, BV_DB2, BV_DB3H, BV_DELTA, BV_EBM, BV_CB1, \
    BV_CB2, BV_EB1, BV_EIB, BV_XSCALE, BV_XBIAS, BV_CKL, BV_XBIAS3, \
    BV_XBIAS2 = range(16)
NBV = 16


def _build(bs, reps=1):
    """Emit the per-core Bass program for a batch shard of `bs` rows.

    reps>1 repeats the recurrence (timing experiments only)."""
    nchunk = bs // CH
    nc = bacc.Bacc(None, target_bir_lowering=False)

    lsT = nc.dram_tensor("lsT", [L, bs], dt.float32r, kind="ExternalInput")
    gtT = nc.dram_tensor("gtT", [L, 4, bs], dt.float32r, kind="ExternalInput")
    epsT = nc.dram_tensor("epsT", [L, Z, bs], dt.float32, kind="ExternalInput")
    W = {k: nc.dram_tensor("w_" + k, list(sh), dt.float32r, kind="ExternalInput")
         for k, sh in _W_SHAPES.items()}
    bvec = nc.dram_tensor("bvec", [H, NBV], dt.float32, kind="ExternalInput")
    gbias = nc.dram_tensor("gbias", [H, L * 4], dt.float32, kind="ExternalInput")
    brow = nc.dram_tensor("brow", [H, L], dt.float32, kind="ExternalInput")
    klT = nc.dram_tensor("klT", [L, Z, bs], dt.float32, kind="ExternalOutput")
    tbT = nc.dram_tensor("tbT", [L, 4, bs], dt.float32, kind="ExternalOutput")

    with tile.TileContext(nc) as tc:
        with tc.tile_pool(name="const", bufs=1) as cp, \
             tc.tile_pool(name="state", bufs=1) as sp, \
             tc.tile_pool(name="work", bufs=2) as wp, \
             tc.tile_pool(name="ps", bufs=2, space="PSUM") as pp, \
             tc.tile_pool(name="psD", bufs=1, space="PSUM") as pd, \
             tc.tile_pool(name="psC", bufs=1, space="PSUM") as pc, \
             tc.tile_pool(name="psF", bufs=1, space="PSUM") as pf, \
             tc.tile_pool(name="psG", bufs=3, space="PSUM") as pg, \
             tc.tile_pool(name="dram", bufs=1, space="DRAM") as dp:

            # ---- constants into SBUF (weights rounded to f32r via gpsimd cast DMA)
            wt = {}
            for k, sh in _W_SHAPES.items():
                t = cp.tile(list(sh), dt.float32r, tag="w_" + k)
                nc.sync.dma_start(out=t[:], in_=W[k][:])
                wt[k] = t
            bv = cp.tile([H, NBV], dt.float32, tag="bvec")
            nc.sync.dma_start(out=bv[:], in_=bvec[:])
            gb = cp.tile([H, L * 4], dt.float32, tag="gbias")
            nc.sync.dma_start(out=gb[:], in_=gbias[:])
            br = cp.tile([H, L], dt.float32, tag="brow")
            nc.sync.dma_start(out=br[:], in_=brow[:])

            # ---- persistent state, one tile per chunk (independent dep chains)
            hS = [sp.tile([H, CH], dt.float32r, tag=f"hS{c}", name=f"hS{c}")
                  for c in range(nchunk)]
            cS = [sp.tile([H, CH], dt.float32, tag=f"cS{c}", name=f"cS{c}")
                  for c in range(nchunk)]
            c1 = [sp.tile([H, CH], dt.float32, tag=f"c1{c}", name=f"c1{c}")
                  for c in range(nchunk)]
            for c in range(nchunk):
                nc.vector.memset(hS[c][:].bitcast(dt.float32), 0.0)
                nc.vector.memset(cS[c][:], 0.0)

            einter_d = dp.tile([L, Z, bs], dt.float32r, tag="einter")

            # ---- hoist: i1 -> c1 (label branch)
            for c in range(nchunk):
                sl = slice(c * CH, (c + 1) * CH)
                lst = wp.tile([L, CH], dt.float32r, tag="lst")
                nc.sync.dma_start(out=lst[:], in_=lsT[:, sl])
                pA = pp.tile([H, CH], dt.float32, tag="ps")
                nc.tensor.matmul(pA[:], wt["cw1"][:], lst[:], start=True, stop=True)
                i1a = wp.tile([H, CH], dt.float32r, tag="i1a")
                nc.scalar.activation(i1a[:], pA[:], AF.Relu,
                                     bias=bv[:, BV_CB1:BV_CB1 + 1], scale=1.0)
                pB = pp.tile([H, CH], dt.float32, tag="ps")
                nc.tensor.matmul(pB[:], wt["cw2"][:], i1a[:], start=True, stop=True)
                i1b = wp.tile([H, CH], dt.float32r, tag="i1b")
                nc.scalar.activation(i1b[:], pB[:], AF.Relu,
                                     bias=bv[:, BV_CB2:BV_CB2 + 1], scale=1.0)
                pD = pp.tile([H, CH], dt.float32, tag="ps")
                nc.tensor.matmul(pD[:], wt["cwo1"][:], i1b[:], start=True, stop=True)
                nc.vector.tensor_copy(c1[c][:], pD[:])

            # ---- hoist: einter (gt-only encoder prefix), via DRAM
            for t in range(L):
                for c in range(nchunk):
                    sl = slice(c * CH, (c + 1) * CH)
                    gtc = wp.tile([4, CH], dt.float32r, tag="gtc")
                    nc.sync.dma_start(out=gtc[:], in_=gtT[t, :, sl])
                    pJ = pp.tile([H, CH], dt.float32, tag="ps")
                    nc.tensor.matmul(pJ[:], wt["ew1"][:], gtc[:], start=True, stop=True)
                    e1 = wp.tile([H, CH], dt.float32r, tag="e1")
                    nc.scalar.activation(e1[:], pJ[:], AF.Relu,
                                         bias=bv[:, BV_EB1:BV_EB1 + 1], scale=1.0)
                    pL = pc.tile([64, CH], dt.float32, tag="psC")
                    nc.tensor.matmul(pL[0:Z, :], wt["ew23a"][:], e1[:],
                                     start=True, stop=True)
                    ei = wp.tile([Z, CH], dt.float32r, tag="ei")
                    nc.vector.tensor_scalar(ei[:], pL[0:Z, :],
                                            bv[:Z, BV_EIB:BV_EIB + 1], None, ALU.add)
                    nc.sync.dma_start(out=einter_d[t, :, sl], in_=ei[:])

            # ---- recurrence
            for t in [t for _ in range(reps) for t in range(L)]:
                for c in range(nchunk):
                    sl = slice(c * CH, (c + 1) * CH)
                    # i3 = relu(h @ cw5 + cb5)  (h = hS/2 folded into cw5h)
                    pA = pp.tile([H, CH], dt.float32, tag="ps")
                    nc.tensor.matmul(pA[:], wt["cw5h"][:], hS[c][:],
                                     start=True, stop=True)
                    i3 = wp.tile([H, CH], dt.float32r, tag="i3", bufs=3)
                    nc.vector.tensor_scalar(i3[:], pA[:], bv[:, BV_CB5:BV_CB5 + 1],
                                            0.0, ALU.add, op1=ALU.max)
                    # cond = i3 @ cwo3 + brow[t] + c1
                    pB = pp.tile([H, CH], dt.float32, tag="ps")
                    nc.tensor.matmul(pB[:], wt["cwo3"][:], i3[:], start=True, stop=True)
                    cond = wp.tile([H, CH], dt.float32r, tag="cond", bufs=3)
                    nc.vector.scalar_tensor_tensor(cond[:], pB[:], br[:, t:t + 1],
                                                   c1[c][:], ALU.add, ALU.add)
                    # phint = relu([inter_pre + einter ; ph_pre + pb1])
                    # einter is accumulated on the PE via an identity matmul
                    pC = pc.tile([64, CH], dt.float32, tag="psC")
                    nc.tensor.matmul(pC[:], wt["php"][:], cond[:],
                                     start=True, stop=False, skip_group_check=True)
                    eic = wp.tile([Z, CH], dt.float32r, tag="eic")
                    nc.sync.dma_start(out=eic[:], in_=einter_d[t, :, sl])
                    nc.tensor.matmul(pC[0:Z, :], wt["eye32"][:], eic[:],
                                     start=False, stop=True, skip_group_check=True)
                    phint = wp.tile([64, CH], dt.float32r, tag="phint", bufs=3)
                    nc.vector.tensor_scalar(phint[:], pC[:],
                                            bv[:64, BV_PHINT:BV_PHINT + 1],
                                            0.0, ALU.add, op1=ALU.max)
                    # z-block: psD = [d | zv | f1 | zvc] (raw, biases downstream)
                    pD = pd.tile([H, CH], dt.float32, tag="psD")
                    nc.tensor.matmul(pD[:], wt["Wz"][:], phint[:], start=True, stop=True)
                    # zm reuses the psC bank (zm-mm needs phint, so the wait is free)
                    pZm = pc.tile([Z, CH], dt.float32, tag="psC")
                    nc.tensor.matmul(pZm[:], wt["ewm"][:], phint[0:Z, :],
                                     start=True, stop=True)
                    # X64 = [E=.5exp(f1_b); e2'=.5exp(-zvc_b)]; e3 = exp(.5 zv_b)
                    X64 = wp.tile([64, CH], dt.float32, tag="X64")
                    nc.scalar.activation(X64[:], pD[2 * Z:4 * Z, :], AF.Exp,
                                         bias=bv[:64, BV_XBIAS:BV_XBIAS + 1],
                                         scale=bv[:64, BV_XSCALE:BV_XSCALE + 1])
                    e3 = wp.tile([Z, CH], dt.float32, tag="e3")
                    nc.scalar.activation(e3[:], pD[Z:2 * Z, :], AF.Exp,
                                         bias=bv[:Z, BV_XBIAS3:BV_XBIAS3 + 1],
                                         scale=0.5)
                    # base-32 lane: d2s, m2, k1, klv all live at partitions 32:64
                    d64 = wp.tile([64, CH], dt.float32, tag="d64")
                    nc.scalar.activation(d64[Z:2 * Z, :], pD[0:Z, :], AF.Square,
                                         bias=bv[:Z, BV_DELTA:BV_DELTA + 1], scale=1.0)
                    k64 = wp.tile([64, CH], dt.float32, tag="k64")
                    nc.vector.scalar_tensor_tensor(k64[Z:2 * Z, :],
                                                   pD[2 * Z:3 * Z, :], -0.5,
                                                   X64[0:Z, :], ALU.mult, ALU.add)
                    m64 = wp.tile([64, CH], dt.float32, tag="m64")
                    nc.gpsimd.tensor_tensor(m64[Z:2 * Z, :], d64[Z:2 * Z, :],
                                            X64[Z:2 * Z, :], ALU.mult)
                    klv = wp.tile([64, CH], dt.float32, tag="klv")
                    nc.gpsimd.tensor_tensor(klv[Z:2 * Z, :], m64[Z:2 * Z, :],
                                            k64[Z:2 * Z, :], ALU.add)
                    nc.sync.dma_start(out=klT[t, :, sl], in_=klv[Z:2 * Z, :])
                    # z = zm_b + e3 * eps
                    epc = wp.tile([Z, CH], dt.float32, tag="epc")
                    nc.sync.dma_start(out=epc[:], in_=epsT[t, :, sl])
                    p1v = wp.tile([Z, CH], dt.float32, tag="p1v")
                    nc.gpsimd.tensor_tensor(p1v[:], e3[:], epc[:], ALU.mult)
                    z_ = wp.tile([Z, CH], dt.float32r, tag="z_")
                    nc.vector.scalar_tensor_tensor(z_[:], pZm[:],
                                                   bv[:Z, BV_EBM:BV_EBM + 1],
                                                   p1v[:], ALU.add, ALU.add)
                    # decoder
                    pE = pp.tile([H, CH], dt.float32, tag="ps")
                    nc.tensor.matmul(pE[:], wt["dw1a"][:], cond[:],
                                     start=True, stop=False)
                    nc.tensor.matmul(pE[:], wt["dw1b"][:], z_[:],
                                     start=False, stop=True)
                    d1 = wp.tile([H, CH], dt.float32r, tag="d1", bufs=3)
                    nc.vector.tensor_scalar(d1[:], pE[:], bv[:, BV_DB1:BV_DB1 + 1],
                                            0.0, ALU.add, op1=ALU.max)
                    pF = pf.tile([64, CH], dt.float32, tag="psF")
                    nc.tensor.matmul(pF[:], wt["dw2"][:], d1[:],
                                     start=True, stop=True)
                    d2dec = wp.tile([64, CH], dt.float32r, tag="d2dec")
                    nc.scalar.activation(d2dec[:], pF[:], AF.Relu,
                                         bias=bv[:64, BV_DB2:BV_DB2 + 1], scale=1.0)
                    # d3 reuses the psF bank (d3-mm needs d2dec, so the wait is free)
                    pD3 = pf.tile([4, CH], dt.float32, tag="psF")
                    nc.tensor.matmul(pD3[:], wt["dw3"][:], d2dec[:],
                                     start=True, stop=True)
                    tb = wp.tile([4, CH], dt.float32r, tag="tb")
                    nc.scalar.activation(tb[:], pD3[:], AF.Tanh,
                                         bias=bv[:4, BV_DB3H:BV_DB3H + 1], scale=0.5)
                    nc.sync.dma_start(out=tbT[t, :, sl], in_=tb[:].bitcast(dt.float32))
                    # LSTM gates (i,f,g,o); sig(x)=.5+.5tanh(x/2) folded
                    tg = []
                    for g in range(4):
                        pG = pg.tile([H, CH], dt.float32, tag="gates")
                        nc.tensor.matmul(pG[:], wt["lrh"][:, g * H:(g + 1) * H],
                                         hS[c][:], start=True, stop=False)
                        nc.tensor.matmul(pG[:], wt["lk4h"][:, g * H:(g + 1) * H],
                                         tb[:], start=False, stop=True)
                        tgt = wp.tile([H, CH], dt.float32, tag=f"tg{g}")
                        col = t * 4 + g
                        nc.scalar.activation(tgt[:], pG[:], AF.Tanh,
                                             bias=gb[:, col:col + 1],
                                             scale=1.0 if g == 2 else 0.5)
                        tg.append(tgt)
                    # c' = .5*(1+tf)c' + (1+ti)tg ; tc = tanh(.5 c');
                    # h' = (1+to)tc  (h'=2h, c'=2c)
                    u1 = wp.tile([H, CH], dt.float32, tag="u1")
                    nc.gpsimd.tensor_tensor(u1[:], tg[1][:], cS[c][:], ALU.mult)
                    nc.gpsimd.tensor_tensor(u1[:], u1[:], cS[c][:], ALU.add)
                    u2 = wp.tile([H, CH], dt.float32, tag="u2")
                    nc.gpsimd.tensor_tensor(u2[:], tg[0][:], tg[2][:], ALU.mult)
                    nc.gpsimd.tensor_tensor(u2[:], u2[:], tg[2][:], ALU.add)
                    nc.vector.scalar_tensor_tensor(cS[c][:], u1[:], 0.5, u2[:],
                                                   ALU.mult, ALU.add)
                    tcil = wp.tile([H, CH], dt.float32, tag="tcil")
                    nc.scalar.activation(tcil[:], cS[c][:], AF.Tanh,
                                         bias=0.0, scale=0.5)
                    nc.vector.scalar_tensor_tensor(hS[c][:], tg[3][:], 1.0, tcil[:],
                                                   ALU.add, ALU.mult)

    nc.finalize()
    return nc


def _host_prep(inputs):
    """Fold weights/biases on the host. Returns (weight_map, C_kl)."""
    f4 = np.float32
    g = {k: np.asarray(v, np.float64) for k, v in inputs.items()
         if k not in ("label_set", "bbox_input", "eps")}

    relu = lambda x: np.maximum(x, 0.0)
    eye = np.eye(L)
    i2a = relu(relu(eye @ g["cw3"] + g["cb3"]) @ g["cw4"] + g["cb4"])  # [L,H]
    browA = (i2a @ g["cwo"][H:2 * H] + g["cbo"])                        # [L,H]

    wm = {}
    wm["w_cw1"] = g["cw1"]
    wm["w_cw2"] = g["cw2"]
    wm["w_cwo1"] = g["cwo"][0:H]
    wm["w_cw5h"] = 0.5 * g["cw5"]
    wm["w_cwo3"] = g["cwo"][2 * H:3 * H]
    # phint layout [inter_pre(0:32); ph_pre(32:64)] -> one matmul
    wm["w_php"] = np.concatenate([g["ew3"][H:2 * H], g["pw1"]], axis=1)  # [H,64]
    ZZ = np.zeros((Z, Z))
    # columns: [d | zv | f1 | zvc]; rows: [inter(0:32); ph(32:64)]
    wm["w_Wz"] = np.block([[g["ewm"], g["ewv"], g["ewv"], ZZ],
                           [-g["pwm"], ZZ, -g["pwv"], g["pwv"]]])  # [64, 4Z]
    wm["w_ewm"] = g["ewm"]
    wm["w_dw1a"] = g["dw1"][0:H]
    wm["w_dw1b"] = g["dw1"][H:H + Z]
    wm["w_dw2"] = g["dw2"]
    wm["w_dw3"] = g["dw3"]
    wm["w_lk4h"] = 0.5 * g["lk"][L:L + 4]
    wm["w_lrh"] = 0.5 * g["lr"]
    wm["w_ew1"] = g["ew1"]
    wm["w_ew23a"] = g["ew2"] @ g["ew3"][0:H]
    wm["w_eye32"] = np.eye(Z)

    bvec = np.zeros((H, NBV))
    bvec[:, BV_CB5] = g["cb5"]
    bvec[Z:2 * Z, BV_PHINT] = g["pb1"]          # [inter(no bias); ph+pb1]
    bvec[:, BV_DB1] = g["db1"]
    bvec[:64, BV_DB2] = g["db2"]
    bvec[:4, BV_DB3H] = 0.5 * g["db3"]
    bvec[:Z, BV_DELTA] = g["ebm"] - g["pbm"]
    bvec[:Z, BV_EBM] = g["ebm"]
    bvec[:, BV_CB1] = g["cb1"]
    bvec[:, BV_CB2] = g["cb2"]
    bvec[:, BV_EB1] = g["eb1"]
    bvec[:Z, BV_EIB] = g["eb2"] @ g["ew3"][0:H] + g["eb3"]
    kappa = g["ebv"] - g["pbv"]
    # X64 ACT reads [f1; zvc]: scales [+1; -1]
    bvec[0:Z, BV_XSCALE] = 1.0
    bvec[Z:2 * Z, BV_XSCALE] = -1.0
    bvec[0:Z, BV_XBIAS] = kappa + LN_HALF            # E  = exp(f1_raw + k + ln.5)
    bvec[Z:2 * Z, BV_XBIAS] = -g["pbv"] + LN_HALF    # e2' = exp(-zvc - pbv + ln.5)
    bvec[0:Z, BV_XBIAS3] = 0.5 * g["ebv"]            # e3 = exp(.5*zv_raw + .5*ebv)
    # kl constant C = -0.5*kappa - 0.5 is added on the host after gather
    ckl = (-0.5 * kappa - 0.5).astype(f4)

    base = g["lb"][None, :] + g["lk"][0:L] + 0.5 * g["lk"][L:L + 4].sum(0)  # [L,512]
    gbias = np.zeros((H, L * 4))
    for t in range(L):
        for gi in range(4):
            sc = 1.0 if gi == 2 else 0.5
            gbias[:, t * 4 + gi] = sc * base[t, gi * H:(gi + 1) * H]

    wm = {k: np.ascontiguousarray(v, dtype=f4) for k, v in wm.items()}
    wm["bvec"] = np.ascontiguousarray(bvec, dtype=f4)
    wm["gbias"] = np.ascontiguousarray(gbias, dtype=f4)
    wm["brow"] = np.ascontiguousarray(browA.T, dtype=f4)  # [H, L]
    return wm, ckl


class Runner:
    """Compile once, execute many times (for timing)."""

    def __init__(self, nc, n_cores):
        import jax
        from jax.sharding import Mesh, PartitionSpec
        from jax.experimental.shard_map import shard_map
        from concourse import bass2jax

        bass2jax.install_neuronx_cc_hook()
        self.n_cores = n_cores
        partition_name = (nc.partition_id_tensor.name
                          if nc.partition_id_tensor else None)
        in_names, out_names, out_avals, zero_outs = [], [], [], []
        for alloc in nc.m.functions[0].allocations:
            if not isinstance(alloc, mybir.MemoryLocationSet):
                continue
            name = alloc.memorylocations[0].name
            if alloc.kind == "ExternalInput":
                if name != partition_name:
                    in_names.append(name)
            elif alloc.kind == "ExternalOutput":
                shape = tuple(alloc.tensor_shape)
                dtype = mybir.dt.np(alloc.dtype)
                out_names.append(name)
                out_avals.append(jax.core.ShapedArray(shape, dtype))
                zero_outs.append(np.zeros(shape, dtype))
        self.in_names, self.out_names = in_names, out_names
        self.out_avals, self.zero_outs = out_avals, zero_outs
        n_params, n_outs = len(in_names), len(out_names)
        self.n_params = n_params

        def _body(*args):
            operands = list(args)
            if partition_name is not None:
                operands.append(bass2jax.partition_id_tensor())
            outs = bass2jax._bass_exec_p.bind(
                *operands,
                out_avals=tuple(out_avals),
                in_names=tuple(in_names + out_names
                               + ([partition_name] if partition_name else [])),
                out_names=tuple(out_names),
                lowering_input_output_aliases=(),
                sim_require_finite=True,
                sim_require_nnan=True,
                nc=nc,
            )
            return tuple(outs)

        donate = tuple(range(n_params, n_params + n_outs))
        if n_cores == 1:
            self.fn = jax.jit(_body, donate_argnums=donate, keep_unused=True)
            self.mesh = None
        else:
            devices = jax.devices()[:n_cores]
            mesh = Mesh(np.asarray(devices), ("core",))
            in_specs = (PartitionSpec("core"),) * (n_params + n_outs)
            out_specs = (PartitionSpec("core"),) * n_outs
            self.fn = jax.jit(
                shard_map(_body, mesh=mesh, in_specs=in_specs,
                          out_specs=out_specs, check_rep=False),
                donate_argnums=donate, keep_unused=True)
            self.mesh = mesh

    def run(self, in_maps):
        n = self.n_cores
        assert len(in_maps) == n
        concat_in = [np.concatenate([np.asarray(in_maps[c][k])
                                     for c in range(n)], axis=0)
                     for k in self.in_names]
        concat_zeros = [np.zeros((n * z.shape[0], *z.shape[1:]), z.dtype)
                        for z in self.zero_outs]
        out = self.fn(*concat_in, *concat_zeros)
        return [
            {name: np.asarray(out[i]).reshape(n, *self.out_avals[i].shape)[c]
             for i, name in enumerate(self.out_names)}
            for c in range(n)
        ]

    def bench(self, in_maps, iters=5):
        """Device-resident inputs; time only dispatch+execute."""
        import time
        import jax
        from jax.sharding import NamedSharding, PartitionSpec

        n = self.n_cores
        concat_in = [np.concatenate([np.asarray(in_maps[c][k])
                                     for c in range(n)], axis=0)
                     for k in self.in_names]
        if self.mesh is not None:
            sh = NamedSharding(self.mesh, PartitionSpec("core"))
            dev_in = [jax.device_put(a, sh) for a in concat_in]
        else:
            dev_in = [jax.device_put(a) for a in concat_in]
        jax.block_until_ready(dev_in)
        times = []
        for _ in range(iters):
            zeros = [np.zeros((n * z.shape[0], *z.shape[1:]), z.dtype)
                     for z in self.zero_outs]
            if self.mesh is not None:
                dev_zeros = [jax.device_put(a, sh) for a in zeros]
            else:
                dev_zeros = [jax.device_put(a) for a in zeros]
            jax.block_until_ready(dev_zeros)
            t0 = time.perf_counter()
            out = self.fn(*dev_in, *dev_zeros)
            jax.block_until_ready(out)
            times.append(time.perf_counter() - t0)
            del out
        return times


_CACHE = {}


def _get_runner(bs, n_cores):
    key = (bs, n_cores)
    if key not in _CACHE:
        nc = _build(bs)
        _CACHE[key] = Runner(nc, n_cores)
    return _CACHE[key]


def _make_in_maps(inputs, n_cores):
    wm, ckl = _host_prep(inputs)
    f4 = np.float32
    ls = np.asarray(inputs["label_set"], f4)
    bb = np.asarray(inputs["bbox_input"], f4)
    ep = np.asarray(inputs["eps"], f4)
    bs = ls.shape[0] // n_cores
    in_maps = []
    for c in range(n_cores):
        s = slice(c * bs, (c + 1) * bs)
        m = dict(wm)
        m["lsT"] = np.ascontiguousarray(ls[s].T)
        m["gtT"] = np.ascontiguousarray(bb[s].transpose(1, 2, 0))
        m["epsT"] = np.ascontiguousarray(ep[s].transpose(1, 2, 0))
        in_maps.append(m)
    return in_maps, bs, ckl


def _assemble(results, ckl):
    bbox_parts, kl_parts = [], []
    for r in results:
        tb = r["tbT"]                      # [L,4,bs]
        kl = r["klT"]                      # [L,Z,bs]
        bbox_parts.append((0.5 + 0.5 * tb).transpose(2, 0, 1))
        kl_parts.append(kl.transpose(2, 0, 1) + ckl[None, None, :])
    return (np.ascontiguousarray(np.concatenate(bbox_parts, 0), dtype=np.float32),
            np.ascontiguousarray(np.concatenate(kl_parts, 0), dtype=np.float32))


def kernel(**inputs):
    n_cores = N_CORES
    in_maps, bs, ckl = _make_in_maps(inputs, n_cores)
    runner = _get_runner(bs, n_cores)
    results = runner.run(in_maps)
    return _assemble(results, ckl)
